# revision 1
# baseline (speedup 1.0000x reference)
"""CorrLookup Trainium2 kernel (merged-record dma_gather design).

Reference op (RAFT-style 1-D correlation pyramid lookup): for each pixel n
(N = B*H*W = 196608) and level i (row width Wi = 256 >> i), sample the
pixel's correlation row at x = disp[n]/2^i + k, k = -4..4, with 1-D linear
interpolation and zeros padding; output (B, 36, H, W).

Key identities: with t = floor(disp), floor(disp/2^i) = t >> i, so ALL four
levels' 10-float windows are determined by u = t >> 3.  The host builds one
256-byte record per (pixel, u) holding the four level slices
[row_i[(t>>i) - delta_i ...]] at fixed offsets OFF, where the per-level
sub-offset delta_i = (t>>i) & (2^(3-i)-1) has range 8/2^i.  The lerp plus
sub-offset fuse into a hat filter: out_i[k] = sum_j relu(1-|a_i-j|) *
rec[OFF_i + k + j], a_i = disp/2^i - (8>>i)*u, with 10/6/4/2 taps.

The gather is dma_gather (the one primitive that packs thousands of
independent descriptors into a single instruction: ~1 us fixed + 0.34 ns
per descriptor, vs ~1 us PER 128-descriptor instruction for indirect
DMA).  Records are 256 B (its minimum element), indices are int16 relative
to a per-call base: block of 1024 pixels * 32 records = 32768 rows =
exactly the int16 range.  Indices are consumed wrapped (idx i at partition
i%16, col i//16, replicated per Q7 core) and written out[i%128, i//128, :],
so pixels map column-major (n = col*128 + p); host transposes accordingly.

Sharding: data-parallel over pixels; core c takes batch b = c.
"""

import numpy as np

P = 128
B, H, W = 8, 96, 256
NLVL = 4
K = 9                    # taps per level
NREC = 32                # records per pixel (u = t>>3)
REC = 128                # record slots (fp16) = 256 B
OFF = [0, 18, 32, 44]    # level slice offsets inside a record
SLC = [18, 14, 12, 10]   # level slice widths
TAPS = [10, 6, 4, 2]     # hat taps per level
WS = [W >> i for i in range(NLVL)]

USE_DMA_GATHER = True


def build_bass(n_pix=B * H * W // 8, use_dma_gather=USE_DMA_GATHER):
    """Single-core SPMD program.
    Inputs: rec [n_pix*NREC, REC] f16, disp_cm [P, n_pix/P] f32 (column-major
    pixels: n = col*128 + p), disp_w [P, n_pix/16] f32 (wrapped+replicated:
    disp_w[c, m] = disp[16*m + c%16]; only used by the dma_gather path).
    Output: outd [NLVL*K, P, n_pix/P] f16 (host maps n = col*128 + p)."""
    import concourse.bass as bass
    import concourse.bacc as bacc
    import concourse.mybir as mybir
    from concourse.tile import TileContext

    f32 = mybir.dt.float32
    f16 = mybir.dt.float16
    i32 = mybir.dt.int32
    i16 = mybir.dt.int16
    Alu = mybir.AluOpType

    tcol = n_pix // P            # 192 pixel columns
    mw = n_pix // 16             # 1536 wrapped cols
    nblk = n_pix // 1024         # 24 gather blocks (1024 pixels each)
    half = tcol // 2

    nc = bacc.Bacc(num_swdge_queues=4)
    rec = nc.declare_dram_parameter("rec", [n_pix * NREC, REC], f16, isOutput=False)
    disp_cm = nc.declare_dram_parameter("disp_cm", [P, tcol], f32, isOutput=False)
    disp_w = nc.declare_dram_parameter("disp_w", [P, mw], f32, isOutput=False)
    outd = nc.declare_dram_parameter("outd", [P, tcol, NLVL * K], f16, isOutput=True)

    def robust_floor(pool, d_t, cols, tagp):
        """floor for d >= 0, any f32->i32 rounding mode. Returns (i32, f32)."""
        fi = pool.tile([P, cols], i32, tag=tagp + "fi")
        nc.vector.tensor_copy(out=fi[:], in_=d_t[:])
        ff = pool.tile([P, cols], f32, tag=tagp + "ff")
        nc.vector.tensor_copy(out=ff[:], in_=fi[:])
        er = pool.tile([P, cols], f32, tag=tagp + "er")
        nc.vector.tensor_tensor(out=er[:], in0=d_t[:], in1=ff[:], op=Alu.subtract)
        ng = pool.tile([P, cols], i32, tag=tagp + "ng")
        nc.vector.tensor_scalar(out=ng[:], in0=er[:], scalar1=0.0, scalar2=None,
                                op0=Alu.is_lt)
        nc.vector.tensor_tensor(out=fi[:], in0=fi[:], in1=ng[:], op=Alu.subtract)
        return fi

    with TileContext(nc) as tc:
        with (
            tc.tile_pool(name="keep", bufs=1) as kp,
            tc.tile_pool(name="work", bufs=2) as wp,
            tc.tile_pool(name="res", bufs=2) as rp,
        ):
            g_full = kp.tile([P, tcol, REC], f16)

            if use_dma_gather:
                # ---- wrapped int16 record indices -------------------------
                # r_rel = 512*(m%64) + 32*(c%16) + min(floor(disp_w/8), 31)
                dw = kp.tile([P, mw], f32)
                nc.sync.dma_start(out=dw[:], in_=disp_w[:])
                iom = wp.tile([P, mw], i32, tag="iom")
                nc.gpsimd.iota(iom[:], pattern=[[0, nblk], [512, 64]], base=0,
                               channel_multiplier=0)
                ioc = wp.tile([P, 1], i32, tag="ioc")
                nc.gpsimd.iota(ioc[:], pattern=[[0, 1]], base=0,
                               channel_multiplier=32)
                nc.vector.tensor_scalar(out=ioc[:], in0=ioc[:], scalar1=511,
                                        scalar2=None, op0=Alu.bitwise_and)
                d8 = wp.tile([P, mw], f32, tag="d8")
                nc.scalar.mul(d8[:], dw[:], 0.125)
                uw = robust_floor(wp, d8, mw, "w")
                nc.vector.tensor_scalar(out=uw[:], in0=uw[:], scalar1=31,
                                        scalar2=None, op0=Alu.min)
                nc.vector.tensor_tensor(out=uw[:], in0=uw[:], in1=iom[:], op=Alu.add)
                nc.vector.tensor_tensor(out=uw[:], in0=uw[:],
                                        in1=ioc[:, 0:1].to_broadcast([P, mw]),
                                        op=Alu.add)
                r16 = kp.tile([P, mw], i16)
                nc.vector.tensor_copy(out=r16[:], in_=uw[:])

                # ---- gathers: 24 x 1024 records --------------------------
                for g in range(nblk):
                    nc.gpsimd.dma_gather(
                        out_ap=g_full[:, 8 * g : 8 * (g + 1), :],
                        in_ap=rec[32768 * g : 32768 * (g + 1), :],
                        idxs_ap=r16[:, 64 * g : 64 * (g + 1)],
                        num_idxs=1024, num_idxs_reg=1024, elem_size=REC,
                        single_packet=False, queue_num=g % 4,
                    )

            # ---- per-pixel params (pixel layout) -------------------------
            disp_t = kp.tile([P, tcol], f32)
            nc.sync.dma_start(out=disp_t[:], in_=disp_cm[:])

            if not use_dma_gather:
                # fallback: one indirect DMA per pixel column
                d8p = wp.tile([P, tcol], f32, tag="d8p")
                nc.scalar.mul(d8p[:], disp_t[:], 0.125)
                up = robust_floor(wp, d8p, tcol, "p")
                nc.vector.tensor_scalar(out=up[:], in0=up[:], scalar1=31,
                                        scalar2=None, op0=Alu.min)
                iop = wp.tile([P, tcol], i32, tag="iop")
                nc.gpsimd.iota(iop[:], pattern=[[NREC * P, tcol]], base=0,
                               channel_multiplier=NREC)
                nc.vector.tensor_tensor(out=up[:], in0=up[:], in1=iop[:], op=Alu.add)
                for t in range(tcol):
                    nc.gpsimd.indirect_dma_start(
                        out=g_full[:, t, :],
                        out_offset=None,
                        in_=rec[:],
                        in_offset=bass.IndirectOffsetOnAxis(ap=up[:, t : t + 1],
                                                            axis=0),
                    )

            d8c = wp.tile([P, tcol], f32, tag="d8c")
            nc.scalar.mul(d8c[:], disp_t[:], 0.125)
            u_t = robust_floor(wp, d8c, tcol, "c")
            nc.vector.tensor_scalar(out=u_t[:], in0=u_t[:], scalar1=31,
                                    scalar2=None, op0=Alu.min)
            u_f = kp.tile([P, tcol], f32)
            nc.vector.tensor_copy(out=u_f[:], in_=u_t[:])

            # hat weights per level: h_j = relu(1 - |a_L - j|), fp16
            h_ts = []
            for lvl in range(NLVL):
                taps = TAPS[lvl]
                a_t = wp.tile([P, tcol], f32, tag="a")
                # a = disp*2^-lvl - u_f*(8>>lvl)
                us = wp.tile([P, tcol], f32, tag="us")
                nc.vector.tensor_scalar(out=us[:], in0=u_f[:],
                                        scalar1=float(8 >> lvl), scalar2=None,
                                        op0=Alu.mult)
                dl = wp.tile([P, tcol], f32, tag="dl")
                nc.vector.tensor_scalar(out=dl[:], in0=disp_t[:],
                                        scalar1=1.0 / (1 << lvl), scalar2=None,
                                        op0=Alu.mult)
                nc.vector.tensor_tensor(out=a_t[:], in0=dl[:], in1=us[:],
                                        op=Alu.subtract)
                a16 = wp.tile([P, tcol], f16, tag="a16")
                nc.vector.tensor_copy(out=a16[:], in_=a_t[:])

                ji = wp.tile([P, taps], i32, tag="ji")
                nc.gpsimd.iota(ji[:], pattern=[[1, taps]], base=0,
                               channel_multiplier=0)
                jf = wp.tile([P, taps], f16, tag="jf")
                nc.vector.tensor_copy(out=jf[:], in_=ji[:])

                h_t = kp.tile([P, taps, tcol], f16, tag=f"h{lvl}")
                nc.vector.tensor_tensor(
                    out=h_t[:],
                    in0=a16[:, None, :].to_broadcast([P, taps, tcol]),
                    in1=jf[:, :, None].to_broadcast([P, taps, tcol]),
                    op=Alu.subtract)
                # h = max(0, min(1 + amj, 1 - amj)) = relu(1 - |a - j|)
                hv = wp.tile([P, taps, tcol], f16, tag="hv")
                nc.vector.tensor_scalar(out=hv[:], in0=h_t[:], scalar1=-1.0,
                                        scalar2=1.0, op0=Alu.mult, op1=Alu.add)
                nc.vector.tensor_scalar(out=h_t[:], in0=h_t[:], scalar1=1.0,
                                        scalar2=None, op0=Alu.add)
                nc.vector.tensor_tensor(out=h_t[:], in0=h_t[:], in1=hv[:],
                                        op=Alu.min)
                nc.vector.tensor_scalar(out=h_t[:], in0=h_t[:], scalar1=0.0,
                                        scalar2=None, op0=Alu.max)
                h_ts.append(h_t)

            # ---- interp: res36[p,c,9*lvl+k] = sum_j h_j * rec[OFF+k+j] ----
            res36 = kp.tile([P, tcol, NLVL * K], f16)
            for hf in range(2):
                sl = slice(hf * half, (hf + 1) * half)
                for lvl in range(NLVL):
                    taps, off = TAPS[lvl], OFF[lvl]
                    dst = res36[:, sl, K * lvl : K * (lvl + 1)]
                    tmp_t = rp.tile([P, half, K], f16, tag="tmp")
                    for j in range(taps):
                        gj = g_full[:, sl, off + j : off + j + K]
                        hb = (h_ts[lvl][:, j, sl, None]
                              .to_broadcast([P, half, K]))
                        if j == 0:
                            nc.vector.tensor_tensor(out=dst, in0=gj, in1=hb,
                                                    op=Alu.mult)
                        else:
                            nc.vector.tensor_tensor(out=tmp_t[:], in0=gj, in1=hb,
                                                    op=Alu.mult)
                            nc.vector.tensor_tensor(out=dst, in0=dst,
                                                    in1=tmp_t[:], op=Alu.add)
                nc.sync.dma_start(out=outd[:, sl, :], in_=res36[:, sl, :])

    return nc


def _prep_core(corrs_core, n_pix):
    """Merged-record table [n_pix*NREC, REC] f16 for one core."""
    from numpy.lib.stride_tricks import sliding_window_view as swv

    recs = np.zeros((n_pix, NREC, REC), dtype=np.float16)
    strides = [8, 4, 2, 1]
    for i in range(NLVL):
        wi = WS[i]
        padded = np.zeros((n_pix, 4 + wi + 10), dtype=np.float32)
        padded[:, 4 : 4 + wi] = corrs_core[i]
        win = swv(padded, SLC[i], axis=1)[:, :: strides[i]][:, :NREC]
        recs[:, :, OFF[i] : OFF[i] + SLC[i]] = win
    return recs.reshape(n_pix * NREC, REC)


_CACHE = {}


def kernel(corr0, corr1, corr2, corr3, flow):
    """Full-input entry point: shard over 8 cores, run, gather."""
    from concourse.bass_utils import run_bass_kernel_spmd

    n_cores = 8
    n_pix = B * H * W // n_cores
    tcol = n_pix // P

    if "nc" not in _CACHE:
        nc = build_bass(n_pix=n_pix)
        nc.finalize()
        _CACHE["nc"] = nc
    nc = _CACHE["nc"]

    corrs = [
        np.asarray(c, dtype=np.float32).reshape(B * H * W, w)
        for c, w in zip((corr0, corr1, corr2, corr3), WS)
    ]
    flow = np.asarray(flow, dtype=np.float32)
    disp_full = flow[:, 0].reshape(B * H * W)

    in_maps = []
    for c in range(n_cores):
        sl = slice(c * n_pix, (c + 1) * n_pix)
        disp = np.ascontiguousarray(disp_full[sl])
        dw16 = disp.reshape(n_pix // 16, 16).T          # (16, mw)
        in_maps.append({
            "rec": _prep_core([cr[sl] for cr in corrs], n_pix),
            "disp_cm": np.ascontiguousarray(disp.reshape(tcol, P).T),
            "disp_w": np.ascontiguousarray(np.tile(dw16, (8, 1))),
        })

    res = run_bass_kernel_spmd(nc, in_maps, list(range(n_cores)),
                               trace=_CACHE.get("trace", False))
    _CACHE["last_res"] = res
    outs = []
    for c in range(n_cores):
        od = res.results[c]["outd"].reshape(P, tcol, NLVL * K)
        outs.append(np.transpose(od, (2, 1, 0)).reshape(NLVL * K, H, W))
    return np.stack(outs, axis=0).astype(np.float32)



# revision 3
# speedup vs baseline: 1.3881x; 1.3881x over previous
"""CorrLookup Trainium2 kernel, v2 (fine-grained merged-record dma_gather).

Reference op (RAFT-style 1-D correlation pyramid lookup): for each pixel n
(N = B*H*W = 196608) and level i (row width Wi = 256 >> i), sample the
pixel's correlation row at x = disp[n]/2^i + k, k = -4..4, with 1-D linear
interpolation and zeros padding; output (B, 36, H, W).

v2 changes vs v1 (u = t>>3, 32 recs/pixel, 22 hat taps, ~163 us):
  * u = t>>2 (64 records/pixel): with t = floor(disp), t>>1 = 2u + d1 and
    t>>3 = u>>1 exactly, so the per-level hat filters shrink to
    TAPS = [5, 3, 2, 2] (levels 2/3 become pure 2-tap lerps).
  * Records keep 256 B (dma_gather minimum) but only 44 slots are used:
    level slices [13, 11, 10, 10] at OFF = [0, 13, 24, 34].
  * 48 dma_gather calls x 512 descriptors (64 recs/px -> 512 px per int16
    index window), 4 SWDGE queues, index chunks pipelined so gathers start
    a few us into the kernel.
  * floor() via int-cast(x - 0.5) (round-to-nearest assumed; ROBUST_FLOOR
    flag restores the cast-agnostic 5-op floor).
  * Gathered records are repacked per half to a (slot, col) layout on the
    otherwise-idle Activation engine; all interp MACs then have packed
    step-1 f16 operands -> DVE 2x perf mode (the v1 broadcast-on-last-dim
    operands forced 1x).
Sharding: data-parallel over pixels; core c takes batch b = c.
"""

import numpy as np

P = 128
B, H, W = 8, 96, 256
NLVL = 4
K = 9                    # taps per level
NREC = 64                # records per pixel (u = t>>2)
REC = 128                # record slots (fp16) = 256 B
OFF = [0, 13, 24, 34]    # level slice offsets inside a record
SLC = [13, 11, 10, 10]   # level slice widths
TAPS = [5, 3, 2, 2]      # hat taps per level
WS = [W >> i for i in range(NLVL)]

ROBUST_FLOOR = False     # True: cast-rounding-agnostic floor (more DVE ops)


def build_bass(n_pix=B * H * W // 8):
    """Single-core SPMD program.
    Inputs: rec [n_pix*NREC, REC] f16, disp_cm [P, n_pix/P] f32 (column-major
    pixels: n = col*128 + p), disp_w [P, n_pix/16] f32 (wrapped+replicated:
    disp_w[c, m] = disp[16*m + c%16]; used for gather indices).
    Output: outd [P, NLVL*K, n_pix/P] f16 (host maps n = col*128 + p)."""
    import concourse.bass as bass
    import concourse.bacc as bacc
    import concourse.mybir as mybir
    from concourse.tile import TileContext

    f32 = mybir.dt.float32
    f16 = mybir.dt.float16
    i32 = mybir.dt.int32
    i16 = mybir.dt.int16
    Alu = mybir.AluOpType

    tcol = n_pix // P            # 192 pixel columns
    mw = n_pix // 16             # 1536 wrapped cols
    nblk = n_pix // 512          # 48 gather blocks (512 pixels each)
    nchunk = 4                   # index-computation chunks
    mwc = mw // nchunk           # 384 wrapped cols per chunk
    bpc = nblk // nchunk         # 12 gather blocks per chunk
    half = tcol // 2
    USED = OFF[-1] + SLC[-1]     # 44 used record slots

    nc = bacc.Bacc(num_swdge_queues=4)
    rec = nc.declare_dram_parameter("rec", [n_pix * NREC, REC], f16, isOutput=False)
    disp_cm = nc.declare_dram_parameter("disp_cm", [P, tcol], f32, isOutput=False)
    disp_w = nc.declare_dram_parameter("disp_w", [P, mw], f32, isOutput=False)
    outd = nc.declare_dram_parameter("outd", [P, NLVL * K, tcol], f16, isOutput=True)

    def robust_floor(pool, d_t, cols, tagp):
        """floor for d >= 0, any f32->i32 rounding mode. Returns i32."""
        fi = pool.tile([P, cols], i32, tag=tagp + "fi")
        nc.vector.tensor_copy(out=fi[:], in_=d_t[:])
        ff = pool.tile([P, cols], f32, tag=tagp + "ff")
        nc.vector.tensor_copy(out=ff[:], in_=fi[:])
        er = pool.tile([P, cols], f32, tag=tagp + "er")
        nc.vector.tensor_tensor(out=er[:], in0=d_t[:], in1=ff[:], op=Alu.subtract)
        ng = pool.tile([P, cols], i32, tag=tagp + "ng")
        nc.vector.tensor_scalar(out=ng[:], in0=er[:], scalar1=0.0, scalar2=None,
                                op0=Alu.is_lt)
        nc.vector.tensor_tensor(out=fi[:], in0=fi[:], in1=ng[:], op=Alu.subtract)
        return fi

    with TileContext(nc) as tc:
        with (
            tc.tile_pool(name="keep", bufs=1) as kp,
            tc.tile_pool(name="work", bufs=2) as wp,
            tc.tile_pool(name="res", bufs=2) as rp,
        ):
            g_full = kp.tile([P, tcol, REC], f16)

            # ---- static index bases (gpsimd; idle before gathers) --------
            # idx for local pixel l of a block = 64*l + u,
            # l = 16*(m%32) + c%16  ->  base = 1024*(m%32) + 64*(c%16).
            iom = kp.tile([P, mwc], i32)
            nc.gpsimd.iota(iom[:], pattern=[[0, bpc], [1024, 32]], base=0,
                           channel_multiplier=0)
            ioc = kp.tile([P, 1], i32)
            nc.gpsimd.iota(ioc[:], pattern=[[0, 1]], base=0,
                           channel_multiplier=64)
            nc.vector.tensor_scalar(out=ioc[:], in0=ioc[:], scalar1=1023,
                                    scalar2=None, op0=Alu.bitwise_and)
            nc.gpsimd.tensor_tensor(out=iom[:], in0=iom[:],
                                    in1=ioc[:, 0:1].to_broadcast([P, mwc]),
                                    op=Alu.add)
            base16 = kp.tile([P, mwc], i16)
            nc.gpsimd.tensor_copy(out=base16[:], in_=iom[:])

            # tap index field jf16[p, j, c] = j (for hat weights)
            ji = kp.tile([P, TAPS[0], tcol], i32)
            nc.gpsimd.iota(ji[:], pattern=[[1, TAPS[0]], [0, tcol]], base=0,
                           channel_multiplier=0)
            jf16 = kp.tile([P, TAPS[0], tcol], f16)
            nc.gpsimd.tensor_copy(out=jf16[:], in_=ji[:])

            # ---- wrapped int16 record indices + gathers, chunked ---------
            r16s = []
            for ck in range(nchunk):
                sl = slice(ck * mwc, (ck + 1) * mwc)
                dw = wp.tile([P, mwc], f32, tag="dw")
                nc.sync.dma_start(out=dw[:], in_=disp_w[:, sl])
                uw16 = kp.tile([P, mwc], i16, tag=f"uw{ck}")
                if ROBUST_FLOOR:
                    d4 = wp.tile([P, mwc], f32, tag="d4w")
                    nc.scalar.mul(d4[:], dw[:], 0.25)
                    fi = robust_floor(wp, d4, mwc, "w")
                    nc.vector.tensor_copy(out=uw16[:], in_=fi[:])
                else:
                    # u = nearest_int(disp/4 - 0.5) == floor(disp/4) away
                    # from exact integers; at exact integers it may round
                    # low, which the taps cover (a hits its closed upper
                    # bound with zero lerp fraction).
                    d4 = wp.tile([P, mwc], f32, tag="d4w")
                    nc.scalar.activation(d4[:], dw[:],
                                         mybir.ActivationFunctionType.Copy,
                                         bias=-0.5, scale=0.25)
                    nc.vector.tensor_copy(out=uw16[:], in_=d4[:])
                r16 = kp.tile([P, mwc], i16, tag=f"r{ck}")
                nc.vector.tensor_tensor(out=r16[:], in0=uw16[:], in1=base16[:],
                                        op=Alu.add)
                r16s.append(r16)

                for bg in range(bpc):
                    g = ck * bpc + bg
                    nc.gpsimd.dma_gather(
                        out_ap=g_full[:, 4 * g : 4 * (g + 1), :],
                        in_ap=rec[32768 * g : 32768 * (g + 1), :],
                        idxs_ap=r16[:, 32 * bg : 32 * (bg + 1)],
                        num_idxs=512, num_idxs_reg=512, elem_size=REC,
                        single_packet=False, queue_num=g % 4,
                    )

            # ---- per-pixel params (pixel layout) -------------------------
            disp_t = kp.tile([P, tcol], f32)
            nc.sync.dma_start(out=disp_t[:], in_=disp_cm[:])

            d4c = wp.tile([P, tcol], f32, tag="d4c")
            if ROBUST_FLOOR:
                nc.scalar.mul(d4c[:], disp_t[:], 0.25)
                u_i = robust_floor(wp, d4c, tcol, "c")
            else:
                nc.scalar.activation(d4c[:], disp_t[:],
                                     mybir.ActivationFunctionType.Copy,
                                     bias=-0.5, scale=0.25)
                u_i = wp.tile([P, tcol], i32, tag="ui")
                nc.vector.tensor_copy(out=u_i[:], in_=d4c[:])
            u_f = kp.tile([P, tcol], f32)
            nc.vector.tensor_copy(out=u_f[:], in_=u_i[:])
            uh_i = wp.tile([P, tcol], i32, tag="uh")
            nc.vector.tensor_scalar(out=uh_i[:], in0=u_i[:], scalar1=1,
                                    scalar2=None, op0=Alu.logical_shift_right)
            uh_f = kp.tile([P, tcol], f32)
            nc.vector.tensor_copy(out=uh_f[:], in_=uh_i[:])

            # a_lvl = disp/2^lvl - (4>>lvl)*u   (lvl 3: disp/8 - (u>>1))
            a16s = []
            for lvl in range(NLVL):
                dl = wp.tile([P, tcol], f32, tag="dl")
                nc.scalar.mul(dl[:], disp_t[:], 1.0 / (1 << lvl))
                us = wp.tile([P, tcol], f32, tag="us")
                if lvl == 3:
                    nc.vector.tensor_tensor(out=us[:], in0=dl[:], in1=uh_f[:],
                                            op=Alu.subtract)
                else:
                    sc = wp.tile([P, tcol], f32, tag="sc")
                    nc.vector.tensor_scalar(out=sc[:], in0=u_f[:],
                                            scalar1=float(4 >> lvl),
                                            scalar2=None, op0=Alu.mult)
                    nc.vector.tensor_tensor(out=us[:], in0=dl[:], in1=sc[:],
                                            op=Alu.subtract)
                a16 = kp.tile([P, tcol], f16, tag=f"a16_{lvl}")
                nc.vector.tensor_copy(out=a16[:], in_=us[:])
                a16s.append(a16)

            # hat weights for levels 0/1: h_j = relu(1 - |a - j|), fp16
            h_ts = []
            for lvl in range(2):
                taps = TAPS[lvl]
                h_t = kp.tile([P, taps, tcol], f16, tag=f"h{lvl}")
                nc.vector.tensor_tensor(
                    out=h_t[:],
                    in0=a16s[lvl][:, None, :].to_broadcast([P, taps, tcol]),
                    in1=jf16[:, 0:taps, :],
                    op=Alu.subtract)
                hv = wp.tile([P, taps, tcol], f16, tag="hv")
                nc.vector.tensor_scalar(out=hv[:], in0=h_t[:], scalar1=-1.0,
                                        scalar2=1.0, op0=Alu.mult, op1=Alu.add)
                nc.vector.tensor_scalar(out=h_t[:], in0=h_t[:], scalar1=1.0,
                                        scalar2=None, op0=Alu.add)
                nc.vector.tensor_tensor(out=h_t[:], in0=h_t[:], in1=hv[:],
                                        op=Alu.min)
                nc.vector.tensor_scalar(out=h_t[:], in0=h_t[:], scalar1=0.0,
                                        scalar2=None, op0=Alu.max)
                h_ts.append(h_t)
            # levels 2/3: pure lerp, weights (1-a, a)
            fbar16s = []
            for lvl in (2, 3):
                fb = kp.tile([P, tcol], f16, tag=f"fb{lvl}")
                nc.vector.tensor_scalar(out=fb[:], in0=a16s[lvl][:],
                                        scalar1=-1.0, scalar2=1.0,
                                        op0=Alu.mult, op1=Alu.add)
                fbar16s.append(fb)

            # ---- interp: res36[p, 9*lvl+k, c] = sum_j h_j * G[off+k+j, c] -
            res36 = kp.tile([P, NLVL * K, tcol], f16)
            for hf in range(2):
                sl = slice(hf * half, (hf + 1) * half)
                # repack gathered records to (slot, col) on the Act engine
                g_kc = rp.tile([P, USED, half], f16, tag="gkc")
                nc.scalar.copy(out=g_kc[:],
                               in_=g_full[:, sl, 0:USED].transpose([0, 2, 1]))
                tmp_t = rp.tile([P, K, half], f16, tag="tmp")
                for lvl in range(NLVL):
                    taps, off = TAPS[lvl], OFF[lvl]
                    dst = res36[:, K * lvl : K * (lvl + 1), sl]
                    for j in range(taps):
                        gj = g_kc[:, off + j : off + j + K, :]
                        if lvl < 2:
                            hb = (h_ts[lvl][:, j : j + 1, sl]
                                  .to_broadcast([P, K, half]))
                        elif j == 0:
                            hb = (fbar16s[lvl - 2][:, None, sl]
                                  .to_broadcast([P, K, half]))
                        else:
                            hb = (a16s[lvl][:, None, sl]
                                  .to_broadcast([P, K, half]))
                        if j == 0:
                            nc.vector.tensor_tensor(out=dst, in0=gj, in1=hb,
                                                    op=Alu.mult)
                        else:
                            nc.vector.tensor_tensor(out=tmp_t[:], in0=gj,
                                                    in1=hb, op=Alu.mult)
                            nc.vector.tensor_tensor(out=dst, in0=dst,
                                                    in1=tmp_t[:], op=Alu.add)
                nc.sync.dma_start(out=outd[:, :, sl], in_=res36[:, :, sl])

    return nc


def _prep_core(corrs_core, n_pix):
    """Merged-record table [n_pix*NREC, REC] f16 for one core."""
    from numpy.lib.stride_tricks import sliding_window_view as swv

    recs = np.zeros((n_pix, NREC, REC), dtype=np.float16)
    for i in range(NLVL):
        wi = WS[i]
        padded = np.zeros((n_pix, 4 + wi + 10), dtype=np.float32)
        padded[:, 4 : 4 + wi] = corrs_core[i]
        win = swv(padded, SLC[i], axis=1)
        if i < 3:
            stride = 4 >> i
            win = win[:, ::stride][:, :NREC]
        else:
            win = win[:, np.arange(NREC) >> 1]
        recs[:, :, OFF[i] : OFF[i] + SLC[i]] = win
    return recs.reshape(n_pix * NREC, REC)


_CACHE = {}


def kernel(corr0, corr1, corr2, corr3, flow):
    """Full-input entry point: shard over 8 cores, run, gather."""
    from concourse.bass_utils import run_bass_kernel_spmd

    n_cores = 8
    n_pix = B * H * W // n_cores
    tcol = n_pix // P

    if "nc" not in _CACHE:
        nc = build_bass(n_pix=n_pix)
        nc.finalize()
        _CACHE["nc"] = nc
    nc = _CACHE["nc"]

    corrs = [
        np.asarray(c, dtype=np.float32).reshape(B * H * W, w)
        for c, w in zip((corr0, corr1, corr2, corr3), WS)
    ]
    flow = np.asarray(flow, dtype=np.float32)
    disp_full = flow[:, 0].reshape(B * H * W)

    in_maps = []
    for c in range(n_cores):
        sl = slice(c * n_pix, (c + 1) * n_pix)
        disp = np.ascontiguousarray(disp_full[sl])
        dw16 = disp.reshape(n_pix // 16, 16).T          # (16, mw)
        in_maps.append({
            "rec": _prep_core([cr[sl] for cr in corrs], n_pix),
            "disp_cm": np.ascontiguousarray(disp.reshape(tcol, P).T),
            "disp_w": np.ascontiguousarray(np.tile(dw16, (8, 1))),
        })

    res = run_bass_kernel_spmd(nc, in_maps, list(range(n_cores)),
                               trace=_CACHE.get("trace", False))
    _CACHE["last_res"] = res
    outs = []
    for c in range(n_cores):
        od = res.results[c]["outd"].reshape(P, NLVL * K, tcol)
        outs.append(np.transpose(od, (1, 2, 0)).reshape(NLVL * K, H, W))
    return np.stack(outs, axis=0).astype(np.float32)


# revision 4
# speedup vs baseline: 1.5609x; 1.1245x over previous
"""CorrLookup Trainium2 kernel, v3 (fine-grained merged-record dma_gather).

Reference op (RAFT-style 1-D correlation pyramid lookup): for each pixel n
(N = B*H*W = 196608) and level i (row width Wi = 256 >> i), sample the
pixel's correlation row at x = disp[n]/2^i + k, k = -4..4, with 1-D linear
interpolation and zeros padding; output (B, 36, H, W).

Design: one 256-B record per (pixel, u) with u = floor(disp)>>2 holds the
four level slices SLC = [13, 11, 10, 10] at OFF; with t = floor(disp),
t>>1 = 2u + d1 and t>>3 = u>>1 exactly, so levels 2/3 are pure 2-tap lerps
and levels 0/1 need 5/3 hat taps (h_j = relu(1 - |a - j|),
a_l = disp/2^l - (4>>l)*u).

The gather is dma_gather (indices int16, relative to a per-call base:
blocks of 512 pixels * 64 records = 32768 rows = the int16 range); it is
descriptor-bound (~2.3 ns/descriptor aggregate over 16 SDMA engines,
~57 us for 24576 descriptors), so everything else is pipelined under it:
  * index chunks -> gathers start ~7 us into the kernel,
  * static iota tables (index base, tap index field) are host constants,
  * per-quarter: records are repacked to (slot, col) on the Act engine,
    interp MACs run on DVE with every operand packed step-1 f16 (2x perf
    mode; a broadcast on the last dim would force 1x), output DMA'd per
    quarter into a contiguous HBM slab.
floor() is int-cast(x - 0.5) (round-to-nearest; ROBUST_FLOOR restores the
cast-agnostic 5-op floor).
Sharding: data-parallel over pixels; core c takes batch b = c.
"""

import numpy as np

P = 128
B, H, W = 8, 96, 256
NLVL = 4
K = 9                    # taps per level
NREC = 64                # records per pixel (u = t>>2)
REC = 128                # record slots (fp16) = 256 B
OFF = [0, 13, 24, 34]    # level slice offsets inside a record
SLC = [13, 11, 10, 10]   # level slice widths
TAPS = [5, 3, 2, 2]      # hat taps per level
WS = [W >> i for i in range(NLVL)]
NQ = 4                   # interp quarters

ROBUST_FLOOR = False     # True: cast-rounding-agnostic floor (more DVE ops)


def build_bass(n_pix=B * H * W // 8):
    """Single-core SPMD program.
    Inputs: rec [n_pix*NREC, REC] f16, disp_cm [P, n_pix/P] f32 (column-major
    pixels: n = col*128 + p), disp_w [P, n_pix/16] f32 (wrapped+replicated:
    disp_w[c, m] = disp[16*m + c%16]), base16 [P, mw/4] i16 (static gather
    index base), jf16 [P, 5, tcol] f16 (static tap index field).
    Output: outd [NQ, P, 36, tcol/NQ] f16 (host maps n = col*128 + p)."""
    import concourse.bass as bass
    import concourse.bacc as bacc
    import concourse.mybir as mybir
    from concourse.tile import TileContext

    f32 = mybir.dt.float32
    f16 = mybir.dt.float16
    i32 = mybir.dt.int32
    i16 = mybir.dt.int16
    Alu = mybir.AluOpType

    tcol = n_pix // P            # 192 pixel columns
    mw = n_pix // 16             # 1536 wrapped cols
    nblk = n_pix // 512          # 48 gather blocks (512 pixels each)
    nchunk = 4                   # index-computation chunks
    mwc = mw // nchunk           # 384 wrapped cols per chunk
    bpc = nblk // nchunk         # 12 gather blocks per chunk
    qcol = tcol // NQ            # 48 pixel columns per interp quarter
    USED = OFF[-1] + SLC[-1]     # 44 used record slots

    nc = bacc.Bacc(num_swdge_queues=4)
    rec = nc.declare_dram_parameter("rec", [n_pix * NREC, REC], f16, isOutput=False)
    disp_cm = nc.declare_dram_parameter("disp_cm", [P, tcol], f32, isOutput=False)
    disp_w = nc.declare_dram_parameter("disp_w", [P, mw], f32, isOutput=False)
    base16d = nc.declare_dram_parameter("base16", [P, mwc], i16, isOutput=False)
    jf16d = nc.declare_dram_parameter("jf16", [P, TAPS[0], tcol], f16,
                                      isOutput=False)
    outd = nc.declare_dram_parameter("outd", [NQ, P, NLVL * K, qcol], f16,
                                     isOutput=True)

    def robust_floor(pool, d_t, cols, tagp):
        """floor for d >= 0, any f32->i32 rounding mode. Returns i32."""
        fi = pool.tile([P, cols], i32, tag=tagp + "fi")
        nc.vector.tensor_copy(out=fi[:], in_=d_t[:])
        ff = pool.tile([P, cols], f32, tag=tagp + "ff")
        nc.vector.tensor_copy(out=ff[:], in_=fi[:])
        er = pool.tile([P, cols], f32, tag=tagp + "er")
        nc.vector.tensor_tensor(out=er[:], in0=d_t[:], in1=ff[:], op=Alu.subtract)
        ng = pool.tile([P, cols], i32, tag=tagp + "ng")
        nc.vector.tensor_scalar(out=ng[:], in0=er[:], scalar1=0.0, scalar2=None,
                                op0=Alu.is_lt)
        nc.vector.tensor_tensor(out=fi[:], in0=fi[:], in1=ng[:], op=Alu.subtract)
        return fi

    with TileContext(nc) as tc:
        with (
            tc.tile_pool(name="keep", bufs=1) as kp,
            tc.tile_pool(name="work", bufs=2) as wp,
            tc.tile_pool(name="res", bufs=2) as rp,
        ):
            g_full = kp.tile([P, tcol, REC], f16)

            base16 = kp.tile([P, mwc], i16)
            nc.sync.dma_start(out=base16[:], in_=base16d[:])
            jf16 = kp.tile([P, TAPS[0], tcol], f16)
            nc.sync.dma_start(out=jf16[:], in_=jf16d[:])

            # ---- wrapped int16 record indices + gathers, chunked ---------
            for ck in range(nchunk):
                sl = slice(ck * mwc, (ck + 1) * mwc)
                dw = wp.tile([P, mwc], f32, tag="dw")
                nc.sync.dma_start(out=dw[:], in_=disp_w[:, sl])
                uw16 = wp.tile([P, mwc], i16, tag="uw")
                if ROBUST_FLOOR:
                    d4 = wp.tile([P, mwc], f32, tag="d4w")
                    nc.vector.tensor_scalar(out=d4[:], in0=dw[:], scalar1=0.25,
                                            scalar2=None, op0=Alu.mult)
                    fi = robust_floor(wp, d4, mwc, "w")
                    nc.vector.tensor_copy(out=uw16[:], in_=fi[:])
                else:
                    # u = nearest_int(disp/4 - 0.5) == floor(disp/4) away
                    # from exact integers; at exact integers it may round
                    # low, which the taps cover (a hits its closed upper
                    # bound with zero lerp fraction).
                    d4 = wp.tile([P, mwc], f32, tag="d4w")
                    nc.vector.tensor_scalar(out=d4[:], in0=dw[:], scalar1=0.25,
                                            scalar2=-0.5, op0=Alu.mult,
                                            op1=Alu.add)
                    nc.vector.tensor_copy(out=uw16[:], in_=d4[:])
                r16 = kp.tile([P, mwc], i16, tag=f"r{ck}")
                nc.vector.tensor_tensor(out=r16[:], in0=uw16[:], in1=base16[:],
                                        op=Alu.add)

                for bg in range(bpc):
                    g = ck * bpc + bg
                    nc.gpsimd.dma_gather(
                        out_ap=g_full[:, 4 * g : 4 * (g + 1), :],
                        in_ap=rec[32768 * g : 32768 * (g + 1), :],
                        idxs_ap=r16[:, 32 * bg : 32 * (bg + 1)],
                        num_idxs=512, num_idxs_reg=512, elem_size=REC,
                        single_packet=False, queue_num=g % 4,
                    )

            # ---- per-pixel params (pixel layout) -------------------------
            disp_t = kp.tile([P, tcol], f32)
            nc.sync.dma_start(out=disp_t[:], in_=disp_cm[:])

            d4c = wp.tile([P, tcol], f32, tag="d4c")
            if ROBUST_FLOOR:
                nc.vector.tensor_scalar(out=d4c[:], in0=disp_t[:], scalar1=0.25,
                                        scalar2=None, op0=Alu.mult)
                u_i = robust_floor(wp, d4c, tcol, "c")
            else:
                nc.vector.tensor_scalar(out=d4c[:], in0=disp_t[:], scalar1=0.25,
                                        scalar2=-0.5, op0=Alu.mult, op1=Alu.add)
                u_i = wp.tile([P, tcol], i32, tag="ui")
                nc.vector.tensor_copy(out=u_i[:], in_=d4c[:])
            u_f = kp.tile([P, tcol], f32)
            nc.vector.tensor_copy(out=u_f[:], in_=u_i[:])
            uh_i = wp.tile([P, tcol], i32, tag="uh")
            nc.vector.tensor_scalar(out=uh_i[:], in0=u_i[:], scalar1=1,
                                    scalar2=None, op0=Alu.logical_shift_right)
            uh_f = kp.tile([P, tcol], f32)
            nc.vector.tensor_copy(out=uh_f[:], in_=uh_i[:])

            # a_lvl = disp/2^lvl - (4>>lvl)*u   (lvl 3: disp/8 - (u>>1))
            a16s = []
            for lvl in range(NLVL):
                dl = wp.tile([P, tcol], f32, tag="dl")
                nc.scalar.mul(dl[:], disp_t[:], 1.0 / (1 << lvl))
                us = wp.tile([P, tcol], f32, tag="us")
                if lvl == 3:
                    nc.vector.tensor_tensor(out=us[:], in0=dl[:], in1=uh_f[:],
                                            op=Alu.subtract)
                else:
                    sc = wp.tile([P, tcol], f32, tag="sc")
                    nc.vector.tensor_scalar(out=sc[:], in0=u_f[:],
                                            scalar1=float(4 >> lvl),
                                            scalar2=None, op0=Alu.mult)
                    nc.vector.tensor_tensor(out=us[:], in0=dl[:], in1=sc[:],
                                            op=Alu.subtract)
                a16 = kp.tile([P, tcol], f16, tag=f"a16_{lvl}")
                nc.vector.tensor_copy(out=a16[:], in_=us[:])
                a16s.append(a16)

            # hat weights for levels 0/1: h_j = relu(1 - |a - j|), fp16
            h_ts = []
            for lvl in range(2):
                taps = TAPS[lvl]
                h_t = kp.tile([P, taps, tcol], f16, tag=f"h{lvl}")
                nc.vector.tensor_tensor(
                    out=h_t[:],
                    in0=a16s[lvl][:, None, :].to_broadcast([P, taps, tcol]),
                    in1=jf16[:, 0:taps, :],
                    op=Alu.subtract)
                hv = wp.tile([P, taps, tcol], f16, tag="hv")
                nc.vector.tensor_scalar(out=hv[:], in0=h_t[:], scalar1=-1.0,
                                        scalar2=1.0, op0=Alu.mult, op1=Alu.add)
                nc.vector.tensor_scalar(out=h_t[:], in0=h_t[:], scalar1=1.0,
                                        scalar2=None, op0=Alu.add)
                nc.vector.tensor_tensor(out=h_t[:], in0=h_t[:], in1=hv[:],
                                        op=Alu.min)
                nc.vector.tensor_scalar(out=h_t[:], in0=h_t[:], scalar1=0.0,
                                        scalar2=None, op0=Alu.max)
                h_ts.append(h_t)
            # levels 2/3: pure lerp, weights (1-a, a)
            fbar16s = []
            for lvl in (2, 3):
                fb = kp.tile([P, tcol], f16, tag=f"fb{lvl}")
                nc.vector.tensor_scalar(out=fb[:], in0=a16s[lvl][:],
                                        scalar1=-1.0, scalar2=1.0,
                                        op0=Alu.mult, op1=Alu.add)
                fbar16s.append(fb)

            # ---- interp per quarter: res[p, 9l+k, c] = sum_j h_j*G[o+k+j,c]
            for q in range(NQ):
                sl = slice(q * qcol, (q + 1) * qcol)
                # repack gathered records to (slot, col) on the Act engine
                g_kc = rp.tile([P, USED, qcol], f16, tag="gkc")
                nc.scalar.copy(out=g_kc[:],
                               in_=g_full[:, sl, 0:USED].transpose([0, 2, 1]))
                res36 = rp.tile([P, NLVL * K, qcol], f16, tag="res")
                tmp_t = rp.tile([P, K, qcol], f16, tag="tmp")
                for lvl in range(NLVL):
                    taps, off = TAPS[lvl], OFF[lvl]
                    dst = res36[:, K * lvl : K * (lvl + 1), :]
                    for j in range(taps):
                        gj = g_kc[:, off + j : off + j + K, :]
                        if lvl < 2:
                            hb = (h_ts[lvl][:, j : j + 1, sl]
                                  .to_broadcast([P, K, qcol]))
                        elif j == 0:
                            hb = (fbar16s[lvl - 2][:, None, sl]
                                  .to_broadcast([P, K, qcol]))
                        else:
                            hb = (a16s[lvl][:, None, sl]
                                  .to_broadcast([P, K, qcol]))
                        if j == 0:
                            nc.vector.tensor_tensor(out=dst, in0=gj, in1=hb,
                                                    op=Alu.mult)
                        else:
                            nc.vector.tensor_tensor(out=tmp_t[:], in0=gj,
                                                    in1=hb, op=Alu.mult)
                            nc.vector.tensor_tensor(out=dst, in0=dst,
                                                    in1=tmp_t[:], op=Alu.add)
                nc.sync.dma_start(out=outd[q], in_=res36[:])

    return nc


def _prep_core(corrs_core, n_pix):
    """Merged-record table [n_pix*NREC, REC] f16 for one core."""
    from numpy.lib.stride_tricks import sliding_window_view as swv

    recs = np.zeros((n_pix, NREC, REC), dtype=np.float16)
    for i in range(NLVL):
        wi = WS[i]
        padded = np.zeros((n_pix, 4 + wi + 10), dtype=np.float32)
        padded[:, 4 : 4 + wi] = corrs_core[i]
        win = swv(padded, SLC[i], axis=1)
        if i < 3:
            stride = 4 >> i
            win = win[:, ::stride][:, :NREC]
        else:
            win = win[:, np.arange(NREC) >> 1]
        recs[:, :, OFF[i] : OFF[i] + SLC[i]] = win
    return recs.reshape(n_pix * NREC, REC)


def _static_tables(n_pix):
    """Host-precomputed iota tables (data-independent)."""
    mwc = n_pix // 16 // 4
    m = np.arange(mwc, dtype=np.int32)
    c = np.arange(P, dtype=np.int32)
    base = (1024 * (m[None, :] % 32) + 64 * (c[:, None] % 16)).astype(np.int16)
    jf = np.broadcast_to(
        np.arange(TAPS[0], dtype=np.float16)[None, :, None],
        (P, TAPS[0], n_pix // P),
    ).copy()
    return base, jf


_CACHE = {}


def kernel(corr0, corr1, corr2, corr3, flow):
    """Full-input entry point: shard over 8 cores, run, gather."""
    from concourse.bass_utils import run_bass_kernel_spmd

    n_cores = 8
    n_pix = B * H * W // n_cores
    tcol = n_pix // P
    qcol = tcol // NQ

    if "nc" not in _CACHE:
        nc = build_bass(n_pix=n_pix)
        nc.finalize()
        _CACHE["nc"] = nc
    nc = _CACHE["nc"]

    corrs = [
        np.asarray(c, dtype=np.float32).reshape(B * H * W, w)
        for c, w in zip((corr0, corr1, corr2, corr3), WS)
    ]
    flow = np.asarray(flow, dtype=np.float32)
    disp_full = flow[:, 0].reshape(B * H * W)
    base16, jf16 = _static_tables(n_pix)

    in_maps = []
    for c in range(n_cores):
        sl = slice(c * n_pix, (c + 1) * n_pix)
        disp = np.ascontiguousarray(disp_full[sl])
        dw16 = disp.reshape(n_pix // 16, 16).T          # (16, mw)
        in_maps.append({
            "rec": _prep_core([cr[sl] for cr in corrs], n_pix),
            "disp_cm": np.ascontiguousarray(disp.reshape(tcol, P).T),
            "disp_w": np.ascontiguousarray(np.tile(dw16, (8, 1))),
            "base16": base16,
            "jf16": jf16,
        })

    res = run_bass_kernel_spmd(nc, in_maps, list(range(n_cores)),
                               trace=_CACHE.get("trace", False))
    _CACHE["last_res"] = res
    outs = []
    for c in range(n_cores):
        od = res.results[c]["outd"].reshape(NQ, P, NLVL * K, qcol)
        # out[ch, n] with n = (q*qcol + col)*128 + p
        oc = od.transpose(2, 0, 3, 1).reshape(NLVL * K, n_pix)
        outs.append(oc.reshape(NLVL * K, H, W))
    return np.stack(outs, axis=0).astype(np.float32)
